# revision 28
# baseline (speedup 1.0000x reference)
"""Trainium2 Bass kernel for nn_AutocorrelationCorrelogram.

For nervegram [B=4, F=50, T=20000, C=2]: 300 periodic-Hann-windowed frames
of length 512 per (b,f,c) signal, circular autocorrelation via
Wiener-Khinchin (rfft -> |.|^2 -> irfft), relu, normalize by sqrt(zero
lag), keep 256 lags, mean over channels -> [4, 50, 300, 256].

Sharding: pure data parallel over the 200 (b,f) pairs -> 25 per core x 8
cores (SPMD, no collectives).

Per core: 30 superbatches of 10 frames, channels merged into 500-wide
working tiles (cols = [c0 q0 | c0 q1 | c1 q0 | c1 q1], q = 5-frame group
x 25 bf). 5-stage software pipeline (one stage per superbatch per step)
keeps every engine's critical path short so the PE never idles (idle
drops the PE to a low p-state, doubling matmul time):

  T(i):   DMA frames, 16 PE transposes -> time-major trp_k [128 t, 500]
          radix-2 DIF fold with the window riding the PSUM->SBUF copy:
          u = w_lo(.)trp_klo (ACT), v = w_hi(.)trp_khi (DVE), g_e/g_o =
          u +/- v (DVE bf16 2x). Halves the rfft matmul count.
  R(i-1): 8 bf16 matmuls: even bins = 256-DFT of g_e (E1|E2 psum pair),
          odd bins = twiddled DFT of g_o (O1|O2); merged ACT Square per
          pair -> sq tiles (f32r)
  I(i-2): irfft flipped (D stationary, squares moving): acf^T [lags,
          cols]; P = Re^2+Im^2 add folded into PSUM accumulation via
          split DE1/DE2 matrices (bin 256 rides DE2 row 0; channel-mean
          0.5 folded into alpha=0.25); relu -> bf16 (ACT h0 / DVE h1)
  Btr(i-3): 8 bf16 PE transposes back to [cols, lags] (one PSUM bank)
  N(i-4): ACT Rsqrt(acf0), scale+relu and channel-add on GpSimd, out DMA
Input DMA issue on Sync, output on GpSimd. PSUM: trp 2 + fftE 2 +
fftO 2 + acfT 1 + trb 1 = 8 banks exactly.
"""

import sys

import numpy as np

sys.path.insert(0, "/opt/trn_rl_repo")

B, F, T, C = 4, 50, 20000, 2
NUM_FRAME = 300
LEN_FRAME = 512
LAGS = 256
N_CORES = 8
BF_PER_CORE = (B * F) // N_CORES  # 25

FRAMES_PER_SB = 10
N_SB_FULL = NUM_FRAME // FRAMES_PER_SB  # 30
NCOLS = 500  # 2c x 2q x 125

STARTS = np.linspace(0, T - LEN_FRAME, NUM_FRAME).astype(np.int64)


def build_weights():
    t = np.arange(256, dtype=np.float64)
    j = np.arange(128, dtype=np.float64)
    l = np.arange(LAGS, dtype=np.float64)
    tf = np.arange(LEN_FRAME, dtype=np.float64)
    w = 0.5 - 0.5 * np.cos(2.0 * np.pi * tf / LEN_FRAME)  # periodic hann

    # rfft of g_e (256-pt DFT, even bins 2j) / g_o (twiddled, odd bins 2j+1)
    we = np.zeros((256, 256))
    we[:, 0:128] = np.cos(2.0 * np.pi * np.outer(t, j) / 256.0)
    we[:, 128] = (-1.0) ** t  # Re X[256]
    we[:, 129:256] = -np.sin(2.0 * np.pi * np.outer(t, j[1:]) / 256.0)
    wo = np.zeros((256, 256))
    wo[:, 0:128] = np.cos(2.0 * np.pi * np.outer(t, 2 * j + 1) / 512.0)
    wo[:, 128:256] = -np.sin(2.0 * np.pi * np.outer(t, 2 * j + 1) / 512.0)

    # irfft (alpha folds the channel mean; output scales with sqrt(alpha))
    alpha = 0.25
    ang = 2.0 * np.pi * np.outer(2 * j, l) / 512.0
    de1 = (alpha / 512.0) * 2.0 * np.cos(ang)
    de1[0] *= 0.5  # bin 0 coef 1
    de2 = (alpha / 512.0) * 2.0 * np.cos(ang)
    de2[0] = (alpha / 512.0) * np.cos(np.pi * l)  # slot 0 carries bin 256
    dok = (alpha / 512.0) * 2.0 * np.cos(2.0 * np.pi * np.outer(2 * j + 1, l) / 512.0)

    f32 = np.float32
    return {
        "wea": we[0:128].astype(f32),
        "web": we[128:256].astype(f32),
        "woa": wo[0:128].astype(f32),
        "wob": wo[128:256].astype(f32),
        "de1": de1.astype(f32),
        "de2": de2.astype(f32),
        "dok": dok.astype(f32),
        "wv": np.ascontiguousarray(w.astype(f32).reshape(4, 128).T),  # [128,4]
        "eye": np.eye(128, dtype=f32),
        "ones": np.ones((1, 128), dtype=f32),
    }


def build_nc(n_sb=N_SB_FULL):
    from contextlib import ExitStack

    import concourse.bacc as bacc
    import concourse.bass as bass
    import concourse.tile as tile
    from concourse import mybir

    f32 = mybir.dt.float32
    f32r = mybir.dt.float32r
    bf16 = mybir.dt.bfloat16
    AF = mybir.ActivationFunctionType
    ALU = mybir.AluOpType

    nc = bacc.Bacc("TRN2", target_bir_lowering=False, debug=False)

    x = nc.dram_tensor("x", [BF_PER_CORE, T, C], f32, kind="ExternalInput").ap()
    wdefs = [("wea", bf16), ("web", bf16), ("woa", bf16), ("wob", bf16),
             ("de1", f32r), ("de2", f32r), ("dok", f32r)]
    wdram = {
        nm: nc.dram_tensor(nm, [128, 256], dt, kind="ExternalInput").ap()
        for nm, dt in wdefs
    }
    wv_d = nc.dram_tensor("wv", [128, 4], f32, kind="ExternalInput").ap()
    eye_d = nc.dram_tensor("eye", [128, 128], f32, kind="ExternalInput").ap()
    eyeh_d = nc.dram_tensor("eyeh", [128, 128], bf16, kind="ExternalInput").ap()
    ones_d = nc.dram_tensor("ones", [1, 128], f32, kind="ExternalInput").ap()
    out = nc.dram_tensor(
        "out", [BF_PER_CORE, NUM_FRAME, LAGS], f32, kind="ExternalOutput"
    ).ap()

    with tile.TileContext(nc) as tc, ExitStack() as ctx:
        consts = ctx.enter_context(tc.tile_pool(name="consts", bufs=1))
        sb_pool = ctx.enter_context(tc.tile_pool(name="work", bufs=1))
        pp = ctx.enter_context(tc.tile_pool(name="ps", bufs=1, space="PSUM"))

        # ---- load constants once (eye first: gates the first transpose) ----
        eye_sb = consts.tile([128, 128], f32, tag="eye")
        nc.sync.dma_start(out=eye_sb[:], in_=eye_d[:])
        wv_sb = consts.tile([128, 4], f32, tag="wv")
        nc.sync.dma_start(out=wv_sb[:], in_=wv_d[:])
        eyeh_sb = consts.tile([128, 128], bf16, tag="eyeh")
        nc.sync.dma_start(out=eyeh_sb[:], in_=eyeh_d[:])
        ones_sb = consts.tile([1, 128], f32, tag="ones")
        nc.sync.dma_start(out=ones_sb[:], in_=ones_d[:])
        wmat = {}
        for nm, dt in wdefs:
            t_ = consts.tile([128, 256], dt, tag=nm)
            nc.sync.dma_start(out=t_[:], in_=wdram[nm][:])
            wmat[nm] = t_
        zero_b = consts.tile([128, 1], f32, tag="zerob")
        nc.vector.memset(zero_b[:], 0.0)
        eps_b = consts.tile([128, 1], f32, tag="epsb")
        nc.vector.memset(eps_b[:], 1e-30)

        def load_sb(s):
            m0 = s * FRAMES_PER_SB
            tiles = []
            for q in range(2):
                ft = sb_pool.tile([125, LEN_FRAME, C], f32, tag="ft", bufs=10)
                mm = 0
                while mm < 5:
                    m = m0 + 5 * q + mm
                    run = 1
                    while (
                        mm + run < 5
                        and STARTS[m + run] - STARTS[m + run - 1]
                        == STARTS[m + 1] - STARTS[m]
                    ):
                        run += 1
                    s0 = int(STARTS[m])
                    step = int(STARTS[m + 1] - STARTS[m]) if run > 1 else 0
                    src_ap = bass.AP(
                        tensor=x.tensor,
                        offset=x.offset + s0 * C,
                        ap=[
                            [step * C, run],
                            [T * C, BF_PER_CORE],
                            [C, LEN_FRAME],
                            [1, C],
                        ],
                    )
                    nc.gpsimd.dma_start(
                        out=ft[25 * mm : 25 * (mm + run)], in_=src_ap
                    )
                    mm += run
                tiles.append(ft)
            return tiles

        PF = 4
        ft_queue = {}
        for s in range(min(PF, n_sb)):
            ft_queue[s] = load_sb(s)

        # cross-step state
        g_t = {}      # s -> (gAe, gAo, gBe, gBo)
        sq_t = {}     # s -> (sqE pair tile, sqO pair tile)
        relu_t = {}   # s -> [reluT_h0, reluT_h1]
        trb_t = {}    # s -> trb psum tile

        # (k_lo, k_hi, w_lo, w_hi)
        WPAIR = [(0, 2, 0, 2), (1, 3, 1, 3)]

        for i in range(n_sb + 4):
            s_f = i          # transpose + fold
            s_r = i - 1      # rfft + squares
            s_i = i - 2      # irfft + relu
            s_t = i - 3      # trback
            s_n = i - 4      # norm + out
            f_v = s_f < n_sb
            r_v = 0 <= s_r < n_sb
            i_v = 0 <= s_i < n_sb
            t_v = 0 <= s_t < n_sb
            n_v = 0 <= s_n < n_sb

            # ---- [1] irfft h0 of s_i ----
            acfp = None
            if i_v:
                sqE, sqO = sq_t[s_i]
                acfp = pp.tile([128, NCOLS], f32, tag="acfT", bufs=1)
                nc.tensor.matmul(acfp[:], wmat["de1"][:, 0:128], sqE[:, 0, :],
                                 start=True, stop=False)
                nc.tensor.matmul(acfp[:], wmat["de2"][:, 0:128], sqE[:, 1, :],
                                 start=False, stop=False)
                nc.tensor.matmul(acfp[:], wmat["dok"][:, 0:128], sqO[:, 0, :],
                                 start=False, stop=False)
                nc.tensor.matmul(acfp[:], wmat["dok"][:, 0:128], sqO[:, 1, :],
                                 start=False, stop=True)
                # [2] plain relu -> bf16 (DVE), releases the acfT bank fast
                rl0 = sb_pool.tile([128, NCOLS], bf16, tag="rl", bufs=4)
                nc.vector.tensor_scalar_max(rl0[:], acfp[:], 0.0)
                relu_t[s_i] = [rl0]

            trp = {}
            fts = None
            if f_v:
                fts = ft_queue.pop(s_f)

            def do_tr(k):
                tp = pp.tile([128, 2, 250], f32, tag="trp", bufs=2)
                for c in range(C):
                    for q in range(2):
                        nc.tensor.transpose(
                            tp[:, c, 125 * q : 125 * q + 125],
                            fts[q][:, 128 * k : 128 * k + 128, c : c + 1],
                            eye_sb[:125, :125],
                        )
                trp[k] = tp

            def do_fold(pair, add_eng):
                k_lo, k_hi, w_lo, w_hi = WPAIR[pair]
                u = sb_pool.tile([128, NCOLS], bf16, tag="uv", bufs=4)
                nc.scalar.activation(
                    u[:], trp[k_lo].rearrange("p c q -> p (c q)"),
                    AF.Copy, bias=0.0, scale=wv_sb[:, w_lo : w_lo + 1],
                )
                v = sb_pool.tile([128, NCOLS], bf16, tag="uv", bufs=4)
                nc.vector.tensor_scalar_mul(
                    v[:], trp[k_hi].rearrange("p c q -> p (c q)"),
                    wv_sb[:, w_hi : w_hi + 1],
                )
                ge = sb_pool.tile([128, NCOLS], bf16, tag="g", bufs=8)
                go = sb_pool.tile([128, NCOLS], bf16, tag="g", bufs=8)
                add_eng.tensor_add(ge[:], u[:], v[:])
                add_eng.tensor_sub(go[:], u[:], v[:])
                return ge, go

            if f_v:
                # [4] transposes k0, k2; fold pair A (adds on DVE)
                do_tr(0)
                do_tr(2)
                gAe, gAo = do_fold(0, nc.vector)
            if r_v:
                # [8] rfft E pair; [9] merged square
                gs = g_t[s_r]
                fftE = pp.tile([128, 2, 512], f32, tag="fftE", bufs=1)
                nc.tensor.matmul(fftE[:, 0, 0:NCOLS], wmat["wea"][:, 0:128],
                                 gs[0][:], start=True, stop=False)
                nc.tensor.matmul(fftE[:, 1, 0:NCOLS], wmat["wea"][:, 128:256],
                                 gs[0][:], start=True, stop=False)
                nc.tensor.matmul(fftE[:, 0, 0:NCOLS], wmat["web"][:, 0:128],
                                 gs[2][:], start=False, stop=True)
                nc.tensor.matmul(fftE[:, 1, 0:NCOLS], wmat["web"][:, 128:256],
                                 gs[2][:], start=False, stop=True)
                sqE = sb_pool.tile([128, 2, NCOLS], f32r, tag="sqE", bufs=2)
                nc.scalar.activation(
                    sqE[:], fftE[:, :, 0:NCOLS], AF.Square, bias=zero_b[:]
                )

            if f_v:
                # [10] transposes k1, k3; [11-12] fold pair B
                do_tr(1)
                do_tr(3)
                gBe, gBo = do_fold(1, nc.vector)
                g_t[s_f] = (gAe, gAo, gBe, gBo)

            if r_v:
                # [13] rfft O pair; [14] merged square
                fftO = pp.tile([128, 2, 512], f32, tag="fftO", bufs=1)
                nc.tensor.matmul(fftO[:, 0, 0:NCOLS], wmat["woa"][:, 0:128],
                                 gs[1][:], start=True, stop=False)
                nc.tensor.matmul(fftO[:, 1, 0:NCOLS], wmat["woa"][:, 128:256],
                                 gs[1][:], start=True, stop=False)
                nc.tensor.matmul(fftO[:, 0, 0:NCOLS], wmat["wob"][:, 0:128],
                                 gs[3][:], start=False, stop=True)
                nc.tensor.matmul(fftO[:, 1, 0:NCOLS], wmat["wob"][:, 128:256],
                                 gs[3][:], start=False, stop=True)
                sqO = sb_pool.tile([128, 2, NCOLS], f32r, tag="sqO", bufs=2)
                nc.scalar.activation(
                    sqO[:], fftO[:, :, 0:NCOLS], AF.Square, bias=zero_b[:]
                )
                sq_t[s_r] = (sqE, sqO)
                del g_t[s_r]

            if n_v:
                # norm factors from trb col 0 (acf zero-lag)
                trb = trb_t.pop(s_n)
                sqcs = []
                for p in range(4):
                    sqc = sb_pool.tile([125, 1], f32, tag="sqc", bufs=8)
                    nc.scalar.activation(
                        sqc[:], trb[:, p, 0:1], AF.Sqrt, bias=eps_b[:125]
                    )
                    sqcs.append(sqc)
                rccs = []
                for p in range(4):
                    rcc = sb_pool.tile([125, 1], f32, tag="rcc", bufs=8)
                    nc.vector.reciprocal(out=rcc[:], in_=sqcs[p][:])
                    rccs.append(rcc)

            if i_v:
                # [16] irfft h1; [17] relu h1 -> bf16 (DVE)
                sqE, sqO = sq_t.pop(s_i)
                nc.tensor.matmul(acfp[:], wmat["de1"][:, 128:256], sqE[:, 0, :],
                                 start=True, stop=False)
                nc.tensor.matmul(acfp[:], wmat["de2"][:, 128:256], sqE[:, 1, :],
                                 start=False, stop=False)
                nc.tensor.matmul(acfp[:], wmat["dok"][:, 128:256], sqO[:, 0, :],
                                 start=False, stop=False)
                nc.tensor.matmul(acfp[:], wmat["dok"][:, 128:256], sqO[:, 1, :],
                                 start=False, stop=True)
                rl1 = sb_pool.tile([128, NCOLS], bf16, tag="rl", bufs=4)
                nc.vector.tensor_scalar_max(rl1[:], acfp[:], 0.0)
                relu_t[s_i].append(rl1)

            if n_v:
                # [15] scale+relu (c0 on ACT), channel add (c1 on DVE), out
                nts = []
                for g in range(2):
                    nt = sb_pool.tile([125, 256], f32, tag="nt", bufs=4)
                    nc.scalar.activation(
                        nt[:], trb[:, g, :], AF.Relu,
                        bias=zero_b[:125], scale=rccs[g][:],
                    )
                    nts.append(nt)
                m0 = s_n * FRAMES_PER_SB
                for g in range(2):
                    mt = sb_pool.tile([125, 256], f32, tag="mt", bufs=4)
                    nc.vector.scalar_tensor_tensor(
                        out=mt[:], in0=trb[:, 2 + g, :],
                        scalar=rccs[2 + g][:], in1=nts[g][:],
                        op0=ALU.mult, op1=ALU.add,
                    )
                    mf = m0 + 5 * g
                    nc.gpsimd.dma_start(
                        out=out[:, mf : mf + 5, :].rearrange(
                            "bf mm l -> mm bf l"
                        ),
                        in_=mt[:],
                    )

            if t_v:
                # [18] trback: normalized [lags, cols] -> [125 f, 2 g, 256
                # lags], channel mean folded into the PSUM accumulation
                # (cols 0:250 = c0, 250:500 = c1)
                rl = relu_t.pop(s_t)
                trb_n = pp.tile([125, 4, 256], bf16, tag="trb", bufs=1)
                for p in range(4):
                    for h in range(2):
                        nc.tensor.transpose(
                            trb_n[:, p, 128 * h : 128 * h + 128],
                            rl[h][:, 125 * p : 125 * p + 125],
                            eyeh_sb[:, :],
                        )
                trb_t[s_t] = trb_n

            if f_v and s_f + PF < n_sb:
                ft_queue[s_f + PF] = load_sb(s_f + PF)

    nc.compile()
    return nc


_NC_CACHE = {}


def _get_nc(n_sb=N_SB_FULL):
    if n_sb not in _NC_CACHE:
        _NC_CACHE[n_sb] = build_nc(n_sb)
    return _NC_CACHE[n_sb]


def make_in_maps(nerv):
    import ml_dtypes

    xs = nerv.reshape(B * F, T, C)
    wts = build_weights()
    bf = ml_dtypes.bfloat16
    base = {
        "wea": wts["wea"].astype(bf), "web": wts["web"].astype(bf),
        "woa": wts["woa"].astype(bf), "wob": wts["wob"].astype(bf),
        "de1": wts["de1"], "de2": wts["de2"], "dok": wts["dok"],
        "wv": wts["wv"], "eye": wts["eye"],
        "eyeh": wts["eye"].astype(bf),
        "ones": wts["ones"],
    }
    return [
        dict(
            base,
            x=np.ascontiguousarray(xs[BF_PER_CORE * i : BF_PER_CORE * (i + 1)]),
        )
        for i in range(N_CORES)
    ]


def kernel(nervegram, trace=False, **_ignored):
    from concourse.bass_utils import run_bass_kernel_spmd

    nerv = np.ascontiguousarray(np.asarray(nervegram, dtype=np.float32))
    assert nerv.shape == (B, F, T, C)
    in_maps = make_in_maps(nerv)
    nc = _get_nc()
    res = run_bass_kernel_spmd(nc, in_maps, list(range(N_CORES)), trace=trace)
    full = np.concatenate([res.results[i]["out"] for i in range(N_CORES)], axis=0)
    out = full.reshape(B, F, NUM_FRAME, LAGS)
    if trace:
        return out, res
    return out


# revision 29
# speedup vs baseline: 1.4682x; 1.4682x over previous
"""Trainium2 Bass kernel for nn_AutocorrelationCorrelogram.

For nervegram [B=4, F=50, T=20000, C=2]: 300 periodic-Hann-windowed frames
of length 512 per (b,f,c) signal, circular autocorrelation via
Wiener-Khinchin (rfft -> |.|^2 -> irfft), relu, normalize by sqrt(zero
lag), keep 256 lags, mean over channels -> [4, 50, 300, 256].

Sharding: pure data parallel over the 200 (b,f) pairs -> 25 per core x 8
cores (SPMD, no collectives).

Per core: 30 superbatches of 10 frames, channels merged into 500-wide
working tiles (cols = [c0 q0 | c0 q1 | c1 q0 | c1 q1], q = 5-frame group
x 25 bf). 5-stage software pipeline (one stage per superbatch per step)
keeps every engine's critical path short so the PE never idles (idle
drops the PE to a low p-state, doubling matmul time):

  T(i):   DMA frames, 16 PE transposes -> time-major trp_k [128 t, 500]
          radix-2 DIF fold with the window riding the PSUM->SBUF copy:
          u = w_lo(.)trp_klo (ACT), v = w_hi(.)trp_khi (DVE), g_e/g_o =
          u +/- v (DVE bf16 2x). Halves the rfft matmul count.
  R(i-1): 8 bf16 matmuls: even bins = 256-DFT of g_e (E1|E2 psum pair),
          odd bins = twiddled DFT of g_o (O1|O2); merged ACT Square per
          pair -> sq tiles (f32r)
  I(i-2): irfft flipped (D stationary, squares moving): acf^T [lags,
          cols]; P = Re^2+Im^2 add folded into PSUM accumulation via
          split DE1/DE2 matrices (bin 256 rides DE2 row 0; channel-mean
          0.5 folded into alpha=0.25); relu -> bf16 (ACT h0 / DVE h1)
  Btr(i-3): 8 bf16 PE transposes back to [cols x 4 chunks, 256 lags]
  N(i-4): per-chunk 1/sqrt(acf0) (ACT sqrt + DVE recip), scale+relu (ACT)
          and channel add (DVE scalar_tensor_tensor), out DMA (GpSimd)
Input DMA issue on GpSimd (SP dynamic DMA is ~3.5us/issue - never use).
PSUM: trp 2 + fftE 2 + fftO 2 + acfT 1 + trb 1 = 8 banks exactly.
"""

import sys

import numpy as np

sys.path.insert(0, "/opt/trn_rl_repo")

B, F, T, C = 4, 50, 20000, 2
NUM_FRAME = 300
LEN_FRAME = 512
LAGS = 256
N_CORES = 8
BF_PER_CORE = (B * F) // N_CORES  # 25

FRAMES_PER_SB = 10
N_SB_FULL = NUM_FRAME // FRAMES_PER_SB  # 30
NCOLS = 500  # 2c x 2q x 125

STARTS = np.linspace(0, T - LEN_FRAME, NUM_FRAME).astype(np.int64)


def build_weights():
    t = np.arange(256, dtype=np.float64)
    j = np.arange(128, dtype=np.float64)
    l = np.arange(LAGS, dtype=np.float64)
    tf = np.arange(LEN_FRAME, dtype=np.float64)
    w = 0.5 - 0.5 * np.cos(2.0 * np.pi * tf / LEN_FRAME)  # periodic hann

    # rfft of g_e (256-pt DFT, even bins 2j) / g_o (twiddled, odd bins 2j+1)
    we = np.zeros((256, 256))
    we[:, 0:128] = np.cos(2.0 * np.pi * np.outer(t, j) / 256.0)
    we[:, 128] = (-1.0) ** t  # Re X[256]
    we[:, 129:256] = -np.sin(2.0 * np.pi * np.outer(t, j[1:]) / 256.0)
    wo = np.zeros((256, 256))
    wo[:, 0:128] = np.cos(2.0 * np.pi * np.outer(t, 2 * j + 1) / 512.0)
    wo[:, 128:256] = -np.sin(2.0 * np.pi * np.outer(t, 2 * j + 1) / 512.0)

    # irfft (alpha folds the channel mean; output scales with sqrt(alpha))
    alpha = 0.25
    ang = 2.0 * np.pi * np.outer(2 * j, l) / 512.0
    de1 = (alpha / 512.0) * 2.0 * np.cos(ang)
    de1[0] *= 0.5  # bin 0 coef 1
    de2 = (alpha / 512.0) * 2.0 * np.cos(ang)
    de2[0] = (alpha / 512.0) * np.cos(np.pi * l)  # slot 0 carries bin 256
    dok = (alpha / 512.0) * 2.0 * np.cos(2.0 * np.pi * np.outer(2 * j + 1, l) / 512.0)

    f32 = np.float32
    return {
        "wea": we[0:128].astype(f32),
        "web": we[128:256].astype(f32),
        "woa": wo[0:128].astype(f32),
        "wob": wo[128:256].astype(f32),
        "de1": de1.astype(f32),
        "de2": de2.astype(f32),
        "dok": dok.astype(f32),
        "wv": np.ascontiguousarray(w.astype(f32).reshape(4, 128).T),  # [128,4]
        "eye": np.eye(128, dtype=f32),
        "ones": np.ones((1, 128), dtype=f32),
    }


def build_nc(n_sb=N_SB_FULL):
    from contextlib import ExitStack

    import concourse.bacc as bacc
    import concourse.bass as bass
    import concourse.tile as tile
    from concourse import mybir

    f32 = mybir.dt.float32
    f32r = mybir.dt.float32r
    bf16 = mybir.dt.bfloat16
    AF = mybir.ActivationFunctionType
    ALU = mybir.AluOpType

    nc = bacc.Bacc("TRN2", target_bir_lowering=False, debug=False)

    x = nc.dram_tensor("x", [BF_PER_CORE, T, C], f32, kind="ExternalInput").ap()
    wdefs = [("wea", bf16), ("web", bf16), ("woa", bf16), ("wob", bf16),
             ("de1", f32r), ("de2", f32r), ("dok", f32r)]
    wdram = {
        nm: nc.dram_tensor(nm, [128, 256], dt, kind="ExternalInput").ap()
        for nm, dt in wdefs
    }
    wv_d = nc.dram_tensor("wv", [128, 4], f32, kind="ExternalInput").ap()
    eye_d = nc.dram_tensor("eye", [128, 128], f32, kind="ExternalInput").ap()
    eyeh_d = nc.dram_tensor("eyeh", [128, 128], bf16, kind="ExternalInput").ap()
    ones_d = nc.dram_tensor("ones", [1, 128], f32, kind="ExternalInput").ap()
    out = nc.dram_tensor(
        "out", [BF_PER_CORE, NUM_FRAME, LAGS], f32, kind="ExternalOutput"
    ).ap()

    with tile.TileContext(nc) as tc, ExitStack() as ctx:
        consts = ctx.enter_context(tc.tile_pool(name="consts", bufs=1))
        sb_pool = ctx.enter_context(tc.tile_pool(name="work", bufs=1))
        pp = ctx.enter_context(tc.tile_pool(name="ps", bufs=1, space="PSUM"))

        # ---- load constants once (eye first: gates the first transpose) ----
        eye_sb = consts.tile([128, 128], f32, tag="eye")
        nc.sync.dma_start(out=eye_sb[:], in_=eye_d[:])
        wv_sb = consts.tile([128, 4], f32, tag="wv")
        nc.sync.dma_start(out=wv_sb[:], in_=wv_d[:])
        eyeh_sb = consts.tile([128, 128], bf16, tag="eyeh")
        nc.sync.dma_start(out=eyeh_sb[:], in_=eyeh_d[:])
        ones_sb = consts.tile([1, 128], f32, tag="ones")
        nc.sync.dma_start(out=ones_sb[:], in_=ones_d[:])
        wmat = {}
        for nm, dt in wdefs:
            t_ = consts.tile([128, 256], dt, tag=nm)
            nc.sync.dma_start(out=t_[:], in_=wdram[nm][:])
            wmat[nm] = t_
        zero_b = consts.tile([128, 1], f32, tag="zerob")
        nc.vector.memset(zero_b[:], 0.0)
        eps_b = consts.tile([128, 1], f32, tag="epsb")
        nc.vector.memset(eps_b[:], 1e-30)

        def load_sb(s):
            m0 = s * FRAMES_PER_SB
            tiles = []
            for q in range(2):
                ft = sb_pool.tile([125, LEN_FRAME, C], f32, tag="ft", bufs=10)
                mm = 0
                while mm < 5:
                    m = m0 + 5 * q + mm
                    run = 1
                    while (
                        mm + run < 5
                        and STARTS[m + run] - STARTS[m + run - 1]
                        == STARTS[m + 1] - STARTS[m]
                    ):
                        run += 1
                    s0 = int(STARTS[m])
                    step = int(STARTS[m + 1] - STARTS[m]) if run > 1 else 0
                    src_ap = bass.AP(
                        tensor=x.tensor,
                        offset=x.offset + s0 * C,
                        ap=[
                            [step * C, run],
                            [T * C, BF_PER_CORE],
                            [C, LEN_FRAME],
                            [1, C],
                        ],
                    )
                    nc.gpsimd.dma_start(
                        out=ft[25 * mm : 25 * (mm + run)], in_=src_ap
                    )
                    mm += run
                tiles.append(ft)
            return tiles

        PF = 4
        ft_queue = {}
        for s in range(min(PF, n_sb)):
            ft_queue[s] = load_sb(s)

        # cross-step state
        g_t = {}      # s -> (gAe, gAo, gBe, gBo)
        sq_t = {}     # s -> (sqE pair tile, sqO pair tile)
        relu_t = {}   # s -> [reluT_h0, reluT_h1]
        trb_t = {}    # s -> trb psum tile

        # (k_lo, k_hi, w_lo, w_hi)
        WPAIR = [(0, 2, 0, 2), (1, 3, 1, 3)]

        for i in range(n_sb + 4):
            s_f = i          # transpose + fold
            s_r = i - 1      # rfft + squares
            s_i = i - 2      # irfft + relu
            s_t = i - 3      # trback
            s_n = i - 4      # norm + out
            f_v = s_f < n_sb
            r_v = 0 <= s_r < n_sb
            i_v = 0 <= s_i < n_sb
            t_v = 0 <= s_t < n_sb
            n_v = 0 <= s_n < n_sb

            # ---- [1] irfft h0 of s_i ----
            acfp = None
            if i_v:
                sqE, sqO = sq_t[s_i]
                acfp = pp.tile([128, NCOLS], f32, tag="acfT", bufs=1)
                nc.tensor.matmul(acfp[:], wmat["de1"][:, 0:128], sqE[:, 0, :],
                                 start=True, stop=False)
                nc.tensor.matmul(acfp[:], wmat["de2"][:, 0:128], sqE[:, 1, :],
                                 start=False, stop=False)
                nc.tensor.matmul(acfp[:], wmat["dok"][:, 0:128], sqO[:, 0, :],
                                 start=False, stop=False)
                nc.tensor.matmul(acfp[:], wmat["dok"][:, 0:128], sqO[:, 1, :],
                                 start=False, stop=True)
                # [2] plain relu -> bf16 (DVE), releases the acfT bank fast
                rl0 = sb_pool.tile([128, NCOLS], bf16, tag="rl", bufs=4)
                nc.vector.tensor_scalar_max(rl0[:], acfp[:], 0.0)
                relu_t[s_i] = [rl0]

            trp = {}
            fts = None
            if f_v:
                fts = ft_queue.pop(s_f)

            def do_tr(k):
                tp = pp.tile([128, 2, 250], f32, tag="trp", bufs=2)
                for c in range(C):
                    for q in range(2):
                        nc.tensor.transpose(
                            tp[:, c, 125 * q : 125 * q + 125],
                            fts[q][:, 128 * k : 128 * k + 128, c : c + 1],
                            eye_sb[:125, :125],
                        )
                trp[k] = tp

            def do_fold(pair, add_eng):
                k_lo, k_hi, w_lo, w_hi = WPAIR[pair]
                u = sb_pool.tile([128, NCOLS], bf16, tag="uv", bufs=4)
                nc.scalar.activation(
                    u[:], trp[k_lo].rearrange("p c q -> p (c q)"),
                    AF.Copy, bias=0.0, scale=wv_sb[:, w_lo : w_lo + 1],
                )
                v = sb_pool.tile([128, NCOLS], bf16, tag="uv", bufs=4)
                nc.vector.tensor_scalar_mul(
                    v[:], trp[k_hi].rearrange("p c q -> p (c q)"),
                    wv_sb[:, w_hi : w_hi + 1],
                )
                ge = sb_pool.tile([128, NCOLS], bf16, tag="g", bufs=8)
                go = sb_pool.tile([128, NCOLS], bf16, tag="g", bufs=8)
                add_eng.tensor_add(ge[:], u[:], v[:])
                add_eng.tensor_sub(go[:], u[:], v[:])
                return ge, go

            if f_v:
                # [4] transposes k0, k2; fold pair A (adds on DVE)
                do_tr(0)
                do_tr(2)
                gAe, gAo = do_fold(0, nc.vector)
            if r_v:
                # [8] rfft E pair; [9] merged square
                gs = g_t[s_r]
                fftE = pp.tile([128, 2, 512], f32, tag="fftE", bufs=1)
                nc.tensor.matmul(fftE[:, 0, 0:NCOLS], wmat["wea"][:, 0:128],
                                 gs[0][:], start=True, stop=False)
                nc.tensor.matmul(fftE[:, 1, 0:NCOLS], wmat["wea"][:, 128:256],
                                 gs[0][:], start=True, stop=False)
                nc.tensor.matmul(fftE[:, 0, 0:NCOLS], wmat["web"][:, 0:128],
                                 gs[2][:], start=False, stop=True)
                nc.tensor.matmul(fftE[:, 1, 0:NCOLS], wmat["web"][:, 128:256],
                                 gs[2][:], start=False, stop=True)
                sqE = sb_pool.tile([128, 2, NCOLS], f32r, tag="sqE", bufs=2)
                nc.scalar.activation(
                    sqE[:], fftE[:, :, 0:NCOLS], AF.Square, bias=zero_b[:]
                )

            if f_v:
                # [10] transposes k1, k3; [11-12] fold pair B
                do_tr(1)
                do_tr(3)
                gBe, gBo = do_fold(1, nc.vector)
                g_t[s_f] = (gAe, gAo, gBe, gBo)

            if r_v:
                # [13] rfft O pair; [14] merged square
                fftO = pp.tile([128, 2, 512], f32, tag="fftO", bufs=1)
                nc.tensor.matmul(fftO[:, 0, 0:NCOLS], wmat["woa"][:, 0:128],
                                 gs[1][:], start=True, stop=False)
                nc.tensor.matmul(fftO[:, 1, 0:NCOLS], wmat["woa"][:, 128:256],
                                 gs[1][:], start=True, stop=False)
                nc.tensor.matmul(fftO[:, 0, 0:NCOLS], wmat["wob"][:, 0:128],
                                 gs[3][:], start=False, stop=True)
                nc.tensor.matmul(fftO[:, 1, 0:NCOLS], wmat["wob"][:, 128:256],
                                 gs[3][:], start=False, stop=True)
                sqO = sb_pool.tile([128, 2, NCOLS], f32r, tag="sqO", bufs=2)
                nc.scalar.activation(
                    sqO[:], fftO[:, :, 0:NCOLS], AF.Square, bias=zero_b[:]
                )
                sq_t[s_r] = (sqE, sqO)
                del g_t[s_r]

            if n_v:
                # norm factors from trb col 0 (acf zero-lag)
                trb = trb_t.pop(s_n)
                sqcs = []
                for p in range(4):
                    sqc = sb_pool.tile([125, 1], f32, tag="sqc", bufs=8)
                    nc.scalar.activation(
                        sqc[:], trb[:, p, 0:1], AF.Sqrt, bias=eps_b[:125]
                    )
                    sqcs.append(sqc)
                rccs = []
                for p in range(4):
                    rcc = sb_pool.tile([125, 1], f32, tag="rcc", bufs=8)
                    nc.vector.reciprocal(out=rcc[:], in_=sqcs[p][:])
                    rccs.append(rcc)

            if i_v:
                # [16] irfft h1; [17] relu h1 -> bf16 (DVE)
                sqE, sqO = sq_t.pop(s_i)
                nc.tensor.matmul(acfp[:], wmat["de1"][:, 128:256], sqE[:, 0, :],
                                 start=True, stop=False)
                nc.tensor.matmul(acfp[:], wmat["de2"][:, 128:256], sqE[:, 1, :],
                                 start=False, stop=False)
                nc.tensor.matmul(acfp[:], wmat["dok"][:, 128:256], sqO[:, 0, :],
                                 start=False, stop=False)
                nc.tensor.matmul(acfp[:], wmat["dok"][:, 128:256], sqO[:, 1, :],
                                 start=False, stop=True)
                rl1 = sb_pool.tile([128, NCOLS], bf16, tag="rl", bufs=4)
                nc.vector.tensor_scalar_max(rl1[:], acfp[:], 0.0)
                relu_t[s_i].append(rl1)

            if n_v:
                # [15] scale+relu (c0 on ACT), channel add (c1 on DVE), out
                nts = []
                for g in range(2):
                    nt = sb_pool.tile([125, 256], f32, tag="nt", bufs=4)
                    nc.scalar.activation(
                        nt[:], trb[:, g, :], AF.Relu,
                        bias=zero_b[:125], scale=rccs[g][:],
                    )
                    nts.append(nt)
                m0 = s_n * FRAMES_PER_SB
                for g in range(2):
                    mt = sb_pool.tile([125, 256], f32, tag="mt", bufs=4)
                    nc.vector.scalar_tensor_tensor(
                        out=mt[:], in0=trb[:, 2 + g, :],
                        scalar=rccs[2 + g][:], in1=nts[g][:],
                        op0=ALU.mult, op1=ALU.add,
                    )
                    mf = m0 + 5 * g
                    nc.gpsimd.dma_start(
                        out=out[:, mf : mf + 5, :].rearrange(
                            "bf mm l -> mm bf l"
                        ),
                        in_=mt[:],
                    )

            if t_v:
                # [18] trback: normalized [lags, cols] -> [125 f, 2 g, 256
                # lags], channel mean folded into the PSUM accumulation
                # (cols 0:250 = c0, 250:500 = c1)
                rl = relu_t.pop(s_t)
                trb_n = pp.tile([125, 4, 256], bf16, tag="trb", bufs=1)
                for p in range(4):
                    for h in range(2):
                        nc.tensor.transpose(
                            trb_n[:, p, 128 * h : 128 * h + 128],
                            rl[h][:, 125 * p : 125 * p + 125],
                            eyeh_sb[:, :],
                        )
                trb_t[s_t] = trb_n

            if f_v and s_f + PF < n_sb:
                ft_queue[s_f + PF] = load_sb(s_f + PF)

    nc.compile()
    return nc


_NC_CACHE = {}


def _get_nc(n_sb=N_SB_FULL):
    if n_sb not in _NC_CACHE:
        _NC_CACHE[n_sb] = build_nc(n_sb)
    return _NC_CACHE[n_sb]


def make_in_maps(nerv):
    import ml_dtypes

    xs = nerv.reshape(B * F, T, C)
    wts = build_weights()
    bf = ml_dtypes.bfloat16
    base = {
        "wea": wts["wea"].astype(bf), "web": wts["web"].astype(bf),
        "woa": wts["woa"].astype(bf), "wob": wts["wob"].astype(bf),
        "de1": wts["de1"], "de2": wts["de2"], "dok": wts["dok"],
        "wv": wts["wv"], "eye": wts["eye"],
        "eyeh": wts["eye"].astype(bf),
        "ones": wts["ones"],
    }
    return [
        dict(
            base,
            x=np.ascontiguousarray(xs[BF_PER_CORE * i : BF_PER_CORE * (i + 1)]),
        )
        for i in range(N_CORES)
    ]


def kernel(nervegram, trace=False, **_ignored):
    from concourse.bass_utils import run_bass_kernel_spmd

    nerv = np.ascontiguousarray(np.asarray(nervegram, dtype=np.float32))
    assert nerv.shape == (B, F, T, C)
    in_maps = make_in_maps(nerv)
    nc = _get_nc()
    res = run_bass_kernel_spmd(nc, in_maps, list(range(N_CORES)), trace=trace)
    full = np.concatenate([res.results[i]["out"] for i in range(N_CORES)], axis=0)
    out = full.reshape(B, F, NUM_FRAME, LAGS)
    if trace:
        return out, res
    return out


# revision 30
# speedup vs baseline: 1.4870x; 1.0128x over previous
"""Trainium2 Bass kernel for nn_AutocorrelationCorrelogram.

For nervegram [B=4, F=50, T=20000, C=2]: 300 periodic-Hann-windowed frames
of length 512 per (b,f,c) signal, circular autocorrelation via
Wiener-Khinchin (rfft -> |.|^2 -> irfft), relu, normalize by sqrt(zero
lag), keep 256 lags, mean over channels -> [4, 50, 300, 256].

Sharding: pure data parallel over the 200 (b,f) pairs -> 25 per core x 8
cores (SPMD, no collectives).

Per core: 30 superbatches of 10 frames, channels merged into 500-wide
working tiles (cols = [c0 q0 | c0 q1 | c1 q0 | c1 q1], q = 5-frame group
x 25 bf). 5-stage software pipeline (one stage per superbatch per step)
keeps every engine's critical path short so the PE never idles (idle
drops the PE to a low p-state, doubling matmul time):

  T(i):   DMA frames, 16 PE transposes -> time-major trp_k [128 t, 500]
          radix-2 DIF fold with the window riding the PSUM->SBUF copy:
          u = w_lo(.)trp_klo (ACT), v = w_hi(.)trp_khi (DVE), g_e/g_o =
          u +/- v (DVE bf16 2x). Halves the rfft matmul count.
  R(i-1): 8 bf16 matmuls: even bins = 256-DFT of g_e (E1|E2 psum pair),
          odd bins = twiddled DFT of g_o (O1|O2); merged ACT Square per
          pair -> sq tiles (f32r)
  I(i-2): irfft flipped (D stationary, squares moving): acf^T [lags,
          cols]; P = Re^2+Im^2 add folded into PSUM accumulation via
          split DE1/DE2 matrices (bin 256 rides DE2 row 0; channel-mean
          0.5 folded into alpha=0.25); relu -> bf16 (ACT h0 / DVE h1)
  Btr(i-3): 8 bf16 PE transposes back to [cols x 4 chunks, 256 lags]
  N(i-4): per-chunk 1/sqrt(acf0) (ACT sqrt + DVE recip), scale+relu (ACT)
          and channel add (DVE scalar_tensor_tensor), out DMA (GpSimd)
Input DMA issue on GpSimd (SP dynamic DMA is ~3.5us/issue - never use).
PSUM: trp 2 + fftE 2 + fftO 2 + acfT 1 + trb 1 = 8 banks exactly.
"""

import sys

import numpy as np

sys.path.insert(0, "/opt/trn_rl_repo")

B, F, T, C = 4, 50, 20000, 2
NUM_FRAME = 300
LEN_FRAME = 512
LAGS = 256
N_CORES = 8
BF_PER_CORE = (B * F) // N_CORES  # 25

FRAMES_PER_SB = 10
N_SB_FULL = NUM_FRAME // FRAMES_PER_SB  # 30
NCOLS = 500  # 2c x 2q x 125

STARTS = np.linspace(0, T - LEN_FRAME, NUM_FRAME).astype(np.int64)


def build_weights():
    t = np.arange(256, dtype=np.float64)
    j = np.arange(128, dtype=np.float64)
    l = np.arange(LAGS, dtype=np.float64)
    tf = np.arange(LEN_FRAME, dtype=np.float64)
    w = 0.5 - 0.5 * np.cos(2.0 * np.pi * tf / LEN_FRAME)  # periodic hann

    # rfft of g_e (256-pt DFT, even bins 2j) / g_o (twiddled, odd bins 2j+1)
    we = np.zeros((256, 256))
    we[:, 0:128] = np.cos(2.0 * np.pi * np.outer(t, j) / 256.0)
    we[:, 128] = (-1.0) ** t  # Re X[256]
    we[:, 129:256] = -np.sin(2.0 * np.pi * np.outer(t, j[1:]) / 256.0)
    wo = np.zeros((256, 256))
    wo[:, 0:128] = np.cos(2.0 * np.pi * np.outer(t, 2 * j + 1) / 512.0)
    wo[:, 128:256] = -np.sin(2.0 * np.pi * np.outer(t, 2 * j + 1) / 512.0)

    # irfft (alpha folds the channel mean; output scales with sqrt(alpha))
    alpha = 0.25
    ang = 2.0 * np.pi * np.outer(2 * j, l) / 512.0
    de1 = (alpha / 512.0) * 2.0 * np.cos(ang)
    de1[0] *= 0.5  # bin 0 coef 1
    de2 = (alpha / 512.0) * 2.0 * np.cos(ang)
    de2[0] = (alpha / 512.0) * np.cos(np.pi * l)  # slot 0 carries bin 256
    dok = (alpha / 512.0) * 2.0 * np.cos(2.0 * np.pi * np.outer(2 * j + 1, l) / 512.0)

    f32 = np.float32
    return {
        "wea": we[0:128].astype(f32),
        "web": we[128:256].astype(f32),
        "woa": wo[0:128].astype(f32),
        "wob": wo[128:256].astype(f32),
        "de1": de1.astype(f32),
        "de2": de2.astype(f32),
        "dok": dok.astype(f32),
        "wv": np.ascontiguousarray(w.astype(f32).reshape(4, 128).T),  # [128,4]
        "eye": np.eye(128, dtype=f32),
        "ones": np.ones((1, 128), dtype=f32),
    }


def build_nc(n_sb=N_SB_FULL):
    from contextlib import ExitStack

    import concourse.bacc as bacc
    import concourse.bass as bass
    import concourse.tile as tile
    from concourse import mybir

    f32 = mybir.dt.float32
    f32r = mybir.dt.float32r
    bf16 = mybir.dt.bfloat16
    AF = mybir.ActivationFunctionType
    ALU = mybir.AluOpType

    nc = bacc.Bacc("TRN2", target_bir_lowering=False, debug=False)

    x = nc.dram_tensor("x", [BF_PER_CORE, T, C], f32, kind="ExternalInput").ap()
    wdefs = [("wea", bf16), ("web", bf16), ("woa", bf16), ("wob", bf16),
             ("de1", f32r), ("de2", f32r), ("dok", f32r)]
    wdram = {
        nm: nc.dram_tensor(nm, [128, 256], dt, kind="ExternalInput").ap()
        for nm, dt in wdefs
    }
    wv_d = nc.dram_tensor("wv", [128, 4], f32, kind="ExternalInput").ap()
    eye_d = nc.dram_tensor("eye", [128, 128], f32, kind="ExternalInput").ap()
    eyeh_d = nc.dram_tensor("eyeh", [128, 128], bf16, kind="ExternalInput").ap()
    ones_d = nc.dram_tensor("ones", [1, 128], f32, kind="ExternalInput").ap()
    out = nc.dram_tensor(
        "out", [BF_PER_CORE, NUM_FRAME, LAGS], f32, kind="ExternalOutput"
    ).ap()

    with tile.TileContext(nc) as tc, ExitStack() as ctx:
        consts = ctx.enter_context(tc.tile_pool(name="consts", bufs=1))
        sb_pool = ctx.enter_context(tc.tile_pool(name="work", bufs=1))
        pp = ctx.enter_context(tc.tile_pool(name="ps", bufs=1, space="PSUM"))

        # ---- load constants once (eye first: gates the first transpose) ----
        eye_sb = consts.tile([128, 128], f32, tag="eye")
        nc.sync.dma_start(out=eye_sb[:], in_=eye_d[:])
        wv_sb = consts.tile([128, 4], f32, tag="wv")
        nc.sync.dma_start(out=wv_sb[:], in_=wv_d[:])
        eyeh_sb = consts.tile([128, 128], bf16, tag="eyeh")
        nc.sync.dma_start(out=eyeh_sb[:], in_=eyeh_d[:])
        ones_sb = consts.tile([1, 128], f32, tag="ones")
        nc.sync.dma_start(out=ones_sb[:], in_=ones_d[:])
        wmat = {}
        for nm, dt in wdefs:
            t_ = consts.tile([128, 256], dt, tag=nm)
            nc.sync.dma_start(out=t_[:], in_=wdram[nm][:])
            wmat[nm] = t_
        zero_b = consts.tile([128, 1], f32, tag="zerob")
        nc.vector.memset(zero_b[:], 0.0)
        eps_b = consts.tile([128, 1], f32, tag="epsb")
        nc.vector.memset(eps_b[:], 1e-30)

        def load_sb(s):
            m0 = s * FRAMES_PER_SB
            tiles = []
            for q in range(2):
                ft = sb_pool.tile([125, LEN_FRAME, C], f32, tag="ft", bufs=10)
                mm = 0
                while mm < 5:
                    m = m0 + 5 * q + mm
                    run = 1
                    while (
                        mm + run < 5
                        and STARTS[m + run] - STARTS[m + run - 1]
                        == STARTS[m + 1] - STARTS[m]
                    ):
                        run += 1
                    s0 = int(STARTS[m])
                    step = int(STARTS[m + 1] - STARTS[m]) if run > 1 else 0
                    src_ap = bass.AP(
                        tensor=x.tensor,
                        offset=x.offset + s0 * C,
                        ap=[
                            [step * C, run],
                            [T * C, BF_PER_CORE],
                            [C, LEN_FRAME],
                            [1, C],
                        ],
                    )
                    nc.gpsimd.dma_start(
                        out=ft[25 * mm : 25 * (mm + run)], in_=src_ap
                    )
                    mm += run
                tiles.append(ft)
            return tiles

        PF = 4
        ft_queue = {}
        for s in range(min(PF, n_sb)):
            ft_queue[s] = load_sb(s)

        # cross-step state
        g_t = {}      # s -> (gAe, gAo, gBe, gBo)
        sq_t = {}     # s -> (sqE pair tile, sqO pair tile)
        relu_t = {}   # s -> [reluT_h0, reluT_h1]
        trb_t = {}    # s -> trb psum tile

        # (k_lo, k_hi, w_lo, w_hi)
        WPAIR = [(0, 2, 0, 2), (1, 3, 1, 3)]

        for i in range(n_sb + 4):
            s_f = i          # transpose + fold
            s_r = i - 1      # rfft + squares
            s_i = i - 2      # irfft + relu
            s_t = i - 3      # trback
            s_n = i - 4      # norm + out
            f_v = s_f < n_sb
            r_v = 0 <= s_r < n_sb
            i_v = 0 <= s_i < n_sb
            t_v = 0 <= s_t < n_sb
            n_v = 0 <= s_n < n_sb

            # ---- [1] irfft h0 of s_i ----
            acfp = None
            if i_v:
                sqE, sqO = sq_t[s_i]
                acfp = pp.tile([128, NCOLS], f32, tag="acfT", bufs=1)
                nc.tensor.matmul(acfp[:], wmat["de1"][:, 0:128], sqE[:, 0, :],
                                 start=True, stop=False)
                nc.tensor.matmul(acfp[:], wmat["de2"][:, 0:128], sqE[:, 1, :],
                                 start=False, stop=False)
                nc.tensor.matmul(acfp[:], wmat["dok"][:, 0:128], sqO[:, 0, :],
                                 start=False, stop=False)
                nc.tensor.matmul(acfp[:], wmat["dok"][:, 0:128], sqO[:, 1, :],
                                 start=False, stop=True)
                # [2] plain relu -> bf16 (DVE), releases the acfT bank fast
                rl0 = sb_pool.tile([128, NCOLS], bf16, tag="rl", bufs=4)
                nc.vector.tensor_scalar_max(rl0[:], acfp[:], 0.0)
                relu_t[s_i] = [rl0]

            trp = {}
            fts = None
            if f_v:
                fts = ft_queue.pop(s_f)

            def do_tr(k):
                tp = pp.tile([128, 2, 250], f32, tag="trp", bufs=2)
                for c in range(C):
                    for q in range(2):
                        nc.tensor.transpose(
                            tp[:, c, 125 * q : 125 * q + 125],
                            fts[q][:, 128 * k : 128 * k + 128, c : c + 1],
                            eye_sb[:125, :125],
                        )
                trp[k] = tp

            def do_fold(pair, add_eng):
                k_lo, k_hi, w_lo, w_hi = WPAIR[pair]
                u = sb_pool.tile([128, NCOLS], bf16, tag="uv", bufs=4)
                nc.scalar.activation(
                    u[:], trp[k_lo].rearrange("p c q -> p (c q)"),
                    AF.Copy, bias=0.0, scale=wv_sb[:, w_lo : w_lo + 1],
                )
                v = sb_pool.tile([128, NCOLS], bf16, tag="uv", bufs=4)
                nc.vector.tensor_scalar_mul(
                    v[:], trp[k_hi].rearrange("p c q -> p (c q)"),
                    wv_sb[:, w_hi : w_hi + 1],
                )
                ge = sb_pool.tile([128, NCOLS], bf16, tag="g", bufs=8)
                go = sb_pool.tile([128, NCOLS], bf16, tag="g", bufs=8)
                add_eng.tensor_add(ge[:], u[:], v[:])
                add_eng.tensor_sub(go[:], u[:], v[:])
                return ge, go

            if f_v:
                # [4] transposes k0, k2; fold pair A (adds on DVE)
                do_tr(0)
                do_tr(2)
                gAe, gAo = do_fold(0, nc.vector)
            if r_v:
                # [8] rfft E pair; [9] merged square
                gs = g_t[s_r]
                fftE = pp.tile([128, 2, 512], f32, tag="fftE", bufs=1)
                nc.tensor.matmul(fftE[:, 0, 0:NCOLS], wmat["wea"][:, 0:128],
                                 gs[0][:], start=True, stop=False)
                nc.tensor.matmul(fftE[:, 1, 0:NCOLS], wmat["wea"][:, 128:256],
                                 gs[0][:], start=True, stop=False)
                nc.tensor.matmul(fftE[:, 0, 0:NCOLS], wmat["web"][:, 0:128],
                                 gs[2][:], start=False, stop=True)
                nc.tensor.matmul(fftE[:, 1, 0:NCOLS], wmat["web"][:, 128:256],
                                 gs[2][:], start=False, stop=True)
                sqE = sb_pool.tile([128, 2, NCOLS], f32r, tag="sqE", bufs=2)
                nc.scalar.activation(
                    sqE[:], fftE[:, :, 0:NCOLS], AF.Square, bias=zero_b[:]
                )

            if f_v:
                # [10] transposes k1, k3; [11-12] fold pair B
                do_tr(1)
                do_tr(3)
                gBe, gBo = do_fold(1, nc.vector)
                g_t[s_f] = (gAe, gAo, gBe, gBo)

            if r_v:
                # [13] rfft O pair; [14] merged square
                fftO = pp.tile([128, 2, 512], f32, tag="fftO", bufs=1)
                nc.tensor.matmul(fftO[:, 0, 0:NCOLS], wmat["woa"][:, 0:128],
                                 gs[1][:], start=True, stop=False)
                nc.tensor.matmul(fftO[:, 1, 0:NCOLS], wmat["woa"][:, 128:256],
                                 gs[1][:], start=True, stop=False)
                nc.tensor.matmul(fftO[:, 0, 0:NCOLS], wmat["wob"][:, 0:128],
                                 gs[3][:], start=False, stop=True)
                nc.tensor.matmul(fftO[:, 1, 0:NCOLS], wmat["wob"][:, 128:256],
                                 gs[3][:], start=False, stop=True)
                sqO = sb_pool.tile([128, 2, NCOLS], f32r, tag="sqO", bufs=2)
                nc.scalar.activation(
                    sqO[:], fftO[:, :, 0:NCOLS], AF.Square, bias=zero_b[:]
                )
                sq_t[s_r] = (sqE, sqO)
                del g_t[s_r]

            if n_v:
                # norm factors from trb col 0 (acf zero-lag), one batched
                # [125,4] sqrt + reciprocal to keep the ACT queue tail short
                trb = trb_t.pop(s_n)
                sqc4 = sb_pool.tile([125, 4], f32, tag="sqc", bufs=4)
                nc.scalar.activation(
                    sqc4[:], trb[:, :, 0:1], AF.Sqrt, bias=eps_b[:125]
                )
                rcc4 = sb_pool.tile([125, 4], f32, tag="rcc", bufs=4)
                nc.vector.reciprocal(out=rcc4[:], in_=sqc4[:])
                rccs = [rcc4[:, p : p + 1] for p in range(4)]

            if i_v:
                # [16] irfft h1; [17] relu h1 -> bf16 (DVE)
                sqE, sqO = sq_t.pop(s_i)
                nc.tensor.matmul(acfp[:], wmat["de1"][:, 128:256], sqE[:, 0, :],
                                 start=True, stop=False)
                nc.tensor.matmul(acfp[:], wmat["de2"][:, 128:256], sqE[:, 1, :],
                                 start=False, stop=False)
                nc.tensor.matmul(acfp[:], wmat["dok"][:, 128:256], sqO[:, 0, :],
                                 start=False, stop=False)
                nc.tensor.matmul(acfp[:], wmat["dok"][:, 128:256], sqO[:, 1, :],
                                 start=False, stop=True)
                rl1 = sb_pool.tile([128, NCOLS], bf16, tag="rl", bufs=4)
                nc.vector.tensor_scalar_max(rl1[:], acfp[:], 0.0)
                relu_t[s_i].append(rl1)

            if n_v:
                # [15] scale+relu (c0 on ACT), channel add (c1 on DVE), out
                nts = []
                for g in range(2):
                    nt = sb_pool.tile([125, 256], f32, tag="nt", bufs=4)
                    nc.scalar.activation(
                        nt[:], trb[:, g, :], AF.Relu,
                        bias=zero_b[:125], scale=rccs[g],
                    )
                    nts.append(nt)
                m0 = s_n * FRAMES_PER_SB
                for g in range(2):
                    mt = sb_pool.tile([125, 256], f32, tag="mt", bufs=4)
                    nc.vector.scalar_tensor_tensor(
                        out=mt[:], in0=trb[:, 2 + g, :],
                        scalar=rccs[2 + g], in1=nts[g][:],
                        op0=ALU.mult, op1=ALU.add,
                    )
                    mf = m0 + 5 * g
                    nc.gpsimd.dma_start(
                        out=out[:, mf : mf + 5, :].rearrange(
                            "bf mm l -> mm bf l"
                        ),
                        in_=mt[:],
                    )

            if t_v:
                # [18] trback: normalized [lags, cols] -> [125 f, 2 g, 256
                # lags], channel mean folded into the PSUM accumulation
                # (cols 0:250 = c0, 250:500 = c1)
                rl = relu_t.pop(s_t)
                trb_n = pp.tile([125, 4, 256], bf16, tag="trb", bufs=1)
                for p in range(4):
                    for h in range(2):
                        nc.tensor.transpose(
                            trb_n[:, p, 128 * h : 128 * h + 128],
                            rl[h][:, 125 * p : 125 * p + 125],
                            eyeh_sb[:, :],
                        )
                trb_t[s_t] = trb_n

            if f_v and s_f + PF < n_sb:
                ft_queue[s_f + PF] = load_sb(s_f + PF)

    nc.compile()
    return nc


_NC_CACHE = {}


def _get_nc(n_sb=N_SB_FULL):
    if n_sb not in _NC_CACHE:
        _NC_CACHE[n_sb] = build_nc(n_sb)
    return _NC_CACHE[n_sb]


def make_in_maps(nerv):
    import ml_dtypes

    xs = nerv.reshape(B * F, T, C)
    wts = build_weights()
    bf = ml_dtypes.bfloat16
    base = {
        "wea": wts["wea"].astype(bf), "web": wts["web"].astype(bf),
        "woa": wts["woa"].astype(bf), "wob": wts["wob"].astype(bf),
        "de1": wts["de1"], "de2": wts["de2"], "dok": wts["dok"],
        "wv": wts["wv"], "eye": wts["eye"],
        "eyeh": wts["eye"].astype(bf),
        "ones": wts["ones"],
    }
    return [
        dict(
            base,
            x=np.ascontiguousarray(xs[BF_PER_CORE * i : BF_PER_CORE * (i + 1)]),
        )
        for i in range(N_CORES)
    ]


def kernel(nervegram, trace=False, **_ignored):
    from concourse.bass_utils import run_bass_kernel_spmd

    nerv = np.ascontiguousarray(np.asarray(nervegram, dtype=np.float32))
    assert nerv.shape == (B, F, T, C)
    in_maps = make_in_maps(nerv)
    nc = _get_nc()
    res = run_bass_kernel_spmd(nc, in_maps, list(range(N_CORES)), trace=trace)
    full = np.concatenate([res.results[i]["out"] for i in range(N_CORES)], axis=0)
    out = full.reshape(B, F, NUM_FRAME, LAGS)
    if trace:
        return out, res
    return out
